# revision 30
# baseline (speedup 1.0000x reference)
"""Trainium2 Bass kernel for nn_MultiHeadAttention (B=1, S=4096, D=2048, H=16, HD=128).

Sharding: tensor-parallel over heads — 2 heads per core on 8 NeuronCores.
Each core computes its 2 heads' Q/K/V projections, causal attention, and a
partial output projection (row-split Wo); the host sums the 8 partials and
adds the output bias (the all-reduce/unshard step).

Layout strategy (per core, all matmuls bf16 with fp32 PSUM accumulation):
  - X^T [2048, 4096] streamed in eight 512-column slices (double-buffered).
    Projections and attention are FUSED: slice e's Q projection is emitted
    first (it gates block e's scores), then its V/K projection units are
    interleaved INTO attention block e's k-group stream as PE fill-work
    while ACT catches up on the exp queue (K/V land before the diagonal
    group, which needs them). Causal attention for block e only needs K/V
    from slices <= e.
  - All inputs are host-reformatted so every tensor loads with ONE
    contiguous 2D DMA (the sync-engine DMA issue rate, ~0.6us/descriptor,
    was the startup bottleneck with per-tile DMAs). The first X slice is split
    into 8 et-pair transfers interleaved with the V-weight chunks so the
    first V matmuls chase the DMA stream from ~1 MB in.
  - Q, K are produced transposed: QT/KT [d, s]. Scores are computed
    transposed, S^T[k, q] = KT_tile^T @ QT, so that p = exp(S^T) tiles have
    k on partitions -> attn@V needs no transpose.
  - Causal masking is multiplicative (0/1) on DVE after the exp. The
    diagonal k-group of each q-block (qb >= 1) is processed triangularly:
    k-tile 4qb+i only computes q columns >= 128i (emitted descending so the
    final full-width matmul carries the PSUM stop flag); the skipped p
    prefix is zeroed so the full-width denominator matmuls stay correct.
    (The 512-free matmul stream advances at ~216ns/instr median with a
    stall tail from exp-latency coupling; scores need the deepest PSUM
    ring available — narrower rings or wider 2-bank tiles regress.)
  - Softmax denominators: ones-column matmuls packed 8 rows into ONE
    PSUM bank (head*64 + 32*par); DVE folds each head's two rows (two
    serial ops — DVE cannot read 2 PSUM operands in one instruction),
    reciprocal_approx_fast inverts, GpSimd partition_broadcast spreads
    1/denom for the normalize multiply. Each head's chain is emitted right
    after its last denominator so it overlaps the other head's last group.
  - O-projection: out[s, e] += outT_h[d, s]^T @ WoT_h[d, e], accumulated over
    both local heads; per s-tile the 4 PSUM results are gathered into one
    [128, 2048] SBUF tile and stored with a single DMA (the last q-block's
    drain copies are split across DVE and the by-then-idle ACT engine).

Build notes:
  - Built with bacc.Bacc: walrus encodes at most ONE sem wait per
    instruction, and Bacc's generate_event_semaphores pass splits larger
    wait sets into event-semaphore chains.
  - PSUM: shared [128,512] pool (projections + scores) bufs=5; one
    accumulator ring (psO h0/h1, psD, then the 16 O-proj psF tiles —
    lifetimes are sequential within a q-block) bufs=3 — exactly 8 banks.
"""

import numpy as np
import ml_dtypes

import concourse.bass as bass
import concourse.mybir as mybir
import concourse.tile as tile
from concourse import bacc
from concourse.bass_utils import run_bass_kernel_spmd


S = 4096          # sequence length
D = 2048          # model dim
NCORES = 8
DL = D // NCORES  # 256 local head dims (2 heads)
NH = 2            # heads per core
HD = 128          # head dim
QB = 512          # q block width
NQB = S // QB     # 8
KT = 128          # k tile (partitions)
NKT = S // KT     # 32
ET = 128          # e contraction tile
NET = D // ET     # 16
NST = S // 128    # 32 s-tiles
SQ = 512          # X^T streaming slice width (s columns)
NSQ = S // SQ     # 8 slices
SCALE = 1.0 / np.sqrt(HD)

BF16 = mybir.dt.bfloat16
F32 = mybir.dt.float32


def build_nc(is_causal: bool) -> bass.Bass:
    # Bacc (not raw Bass): its finalize() pipeline splits multi-sem sync
    # waits into event-semaphore chains — walrus encodes at most one wait
    # per instruction.
    nc = bacc.Bacc()

    # xt2 row-block sl: [128, et*512+c] = X[sl*512+c, et*128+p] (host packed)
    XT2 = nc.dram_tensor("xt2", [NSQ * 128, NET * SQ], BF16, kind="ExternalInput")
    # weights packed [128, et*256+c] = W^T[et*128+p, c]
    WQ2 = nc.dram_tensor("wq2", [128, NET * DL], BF16, kind="ExternalInput")
    WK2 = nc.dram_tensor("wk2", [128, NET * DL], BF16, kind="ExternalInput")
    WV2 = nc.dram_tensor("wv2", [128, NET * DL], BF16, kind="ExternalInput")
    # bias columns [128, 4]: bq.d0 | bq.d1 | bk.d0 | bk.d1
    BQKC = nc.dram_tensor("bqkc", [128, 4], F32, kind="ExternalInput")
    BVROW = nc.dram_tensor("bvrow", [1, DL], BF16, kind="ExternalInput")
    # [128, h*2048+c] = Wo^T[h*128+p, c]
    WO2 = nc.dram_tensor("wo2", [128, NH * D], BF16, kind="ExternalInput")
    # [128, jj*512+q]: multiplicative causal masks (1 below/on diagonal)
    MASKS2 = nc.dram_tensor("masks2", [128, 4 * QB], BF16, kind="ExternalInput")
    OUT = nc.dram_tensor("out", [S, D], F32, kind="ExternalOutput")

    with tile.TileContext(nc) as tc:
        with tc.tile_pool(name="persist", bufs=1) as persist:
            # Q head0 | Q head1 | K head0 | K head1, each [128, 4096]
            qkt = persist.tile([128, 4 * S], BF16, name="qkt")
            # V natural layout: s-tile st at cols [st*256, (st+1)*256), head h at +h*128
            vt = persist.tile([128, NST * DL], BF16, name="vt")
            wot_sb = persist.tile([128, NH * D], BF16, name="wot_sb")
            masks_sb = persist.tile([128, 4 * QB], BF16, name="masks_sb")
            wv_sb = persist.tile([128, NET * DL], BF16, name="wv_sb")
            wk_sb = persist.tile([128, NET * DL], BF16, name="wk_sb")
            wq_sb = persist.tile([128, NET * DL], BF16, name="wq_sb")
            ones_col = persist.tile([128, 1], BF16, name="ones_col")
            biasqk = persist.tile([128, 4], F32, name="biasqk")
            bvrow_sb = persist.tile([1, DL], BF16, name="bvrow_sb")
            bvb_sb = persist.tile([128, DL], BF16, name="bvb_sb")
            # normalized attention outputs, transposed: (h*NQB+qb) tile [128d, 512q]
            outt = persist.tile([128, NH * NQB * QB], BF16, name="outt")

            nc.vector.memset(ones_col[:, :], 1.0)

            with tc.tile_pool(name="xtp", bufs=2) as xtp, \
                 tc.tile_pool(name="ps512", bufs=5, space="PSUM") as ps512, \
                 tc.tile_pool(name="acc4", bufs=3, space="PSUM") as acc4, \
                 tc.tile_pool(name="pp", bufs=14) as pp, \
                 tc.tile_pool(name="rp", bufs=4) as rp, \
                 tc.tile_pool(name="op", bufs=3) as op:

                # DMA order: V weights + X^T slice 0, chunked and interleaved
                # so the first V matmuls start after ~1 MB; then the rest.
                xt_tiles = {}
                def load_xe(sl):
                    xt_e = xtp.tile([128, NET * SQ], BF16, name="xt_e", tag="xt")
                    xt_tiles[sl] = xt_e
                    nc.sync.dma_start(
                        out=xt_e[:, :], in_=XT2[sl * 128 : (sl + 1) * 128, :]
                    )
                xt0 = xtp.tile([128, NET * SQ], BF16, name="xt_e", tag="xt")
                xt_tiles[0] = xt0
                WQC = NET * DL // 4   # weight chunk columns
                XPC = NET * SQ // 8   # x-slice et-pair columns
                for ch in range(4):
                    nc.sync.dma_start(
                        out=wv_sb[:, ch * WQC : (ch + 1) * WQC],
                        in_=WV2[:, ch * WQC : (ch + 1) * WQC],
                    )
                    for half in range(2):
                        pc = 2 * ch + half
                        nc.sync.dma_start(
                            out=xt0[:, pc * XPC : (pc + 1) * XPC],
                            in_=XT2[0:128, pc * XPC : (pc + 1) * XPC],
                        )
                nc.sync.dma_start(out=biasqk[:, :], in_=BQKC[:, :])
                nc.sync.dma_start(out=wq_sb[:, :], in_=WQ2[:, :])
                if is_causal:
                    nc.sync.dma_start(out=masks_sb[:, :], in_=MASKS2[:, :])
                nc.sync.dma_start(out=wk_sb[:, :], in_=WK2[:, :])
                nc.sync.dma_start(out=bvrow_sb[:, :], in_=BVROW[:, :])
                nc.sync.dma_start(out=wot_sb[:, :], in_=WO2[:, :])
                # broadcast bv across partitions once; folded into each V
                # tile's PSUM->SBUF copy below
                nc.gpsimd.partition_broadcast(bvb_sb[:, :], bvrow_sb[:, :])

                def emit_v_tile(sl, stl):
                    xt_e = xt_tiles[sl]
                    st = sl * (SQ // 128) + stl
                    psv = ps512.tile([128, QB], F32, name="psv", tag="ps512")
                    for et in range(NET):
                        nc.tensor.matmul(
                            psv[:, :DL],
                            lhsT=xt_e[:, et * SQ + stl * 128 : et * SQ + (stl + 1) * 128],
                            rhs=wv_sb[:, et * DL : (et + 1) * DL],
                            start=(et == 0),
                            stop=(et == NET - 1),
                        )
                    nc.vector.scalar_tensor_tensor(
                        out=vt[:, st * DL : (st + 1) * DL],
                        in0=psv[:, :DL],
                        scalar=1.0,
                        in1=bvb_sb[:, :],
                        op0=mybir.AluOpType.mult,
                        op1=mybir.AluOpType.add,
                    )

                def emit_qk(sl, w_sb, base4, bias_base, dt):
                    # transposed [d, s] projection for one head; bias add on
                    # ACT (per-partition bias) to keep DVE free
                    xt_e = xt_tiles[sl]
                    psq = ps512.tile([128, QB], F32, name="psq", tag="ps512")
                    for et in range(NET):
                        nc.tensor.matmul(
                            psq[:, :],
                            lhsT=w_sb[:, et * DL + dt * 128 : et * DL + (dt + 1) * 128],
                            rhs=xt_e[:, et * SQ : (et + 1) * SQ],
                            start=(et == 0),
                            stop=(et == NET - 1),
                        )
                    nc.scalar.add(
                        qkt[:, (base4 + dt) * S + sl * QB : (base4 + dt) * S + (sl + 1) * QB],
                        psq[:, :],
                        biasqk[:, bias_base + dt : bias_base + dt + 1],
                    )

                def proj_slice(sl):
                    for stl in range(SQ // 128):
                        emit_v_tile(sl, stl)
                    for w_sb, base4, bias_base in ((wq_sb, 0, 0), (wk_sb, 2, 2)):
                        for dt in range(NH):
                            emit_qk(sl, w_sb, base4, bias_base, dt)

                def attention_qb(qb, units=None):
                    kmax = 4 * (qb + 1) if is_causal else NKT
                    groups = list(range(0, kmax, 4))
                    ngroups = len(groups)
                    # triangular diagonal handling needs a preceding
                    # rectangular group to own the psO start flags
                    tri_diag = is_causal and qb >= 1
                    psO = {}
                    for h in range(NH):
                        psO[h] = acc4.tile([128, QB], F32, name="psO", tag="acc4")
                    # one PSUM bank holds all 8 denominator rows:
                    # row = 64*h + 32*par; two k-tiles per row per group
                    psD = acc4.tile([128, QB], F32, name="psD", tag="acc4")
                    units = list(units) if units else []
                    per_gap = -(-len(units) // max(1, ngroups - 1)) if units else 0
                    for gi, kg in enumerate(groups):
                        diag = is_causal and kg == 4 * qb
                        # interleave projection units into the attention
                        # stream: they give PE fill-work while ACT catches up
                        # on the exp queue. K/V of this slice must land
                        # before the diagonal group.
                        if units and diag:
                            while units:
                                units.pop(0)()
                        elif units and gi > 0:
                            for _ in range(per_gap):
                                if units:
                                    units.pop(0)()
                        for h in range(NH):
                            # (kt, q column offset) pairs; descending kt for
                            # the triangular diagonal so the final matmul is
                            # full-width and carries the stop flag
                            if diag and tri_diag:
                                tiles = [(kg + i, i * 128) for i in (3, 2, 1, 0)]
                            else:
                                tiles = [(kg + i, 0) for i in range(4)]
                            ps_group = []
                            for kt, qo in tiles:
                                psS = ps512.tile([128, QB], F32, name="psS", tag="ps512")
                                nc.tensor.matmul(
                                    psS[:, qo:],
                                    lhsT=qkt[:, (2 + h) * S + kt * 128 : (2 + h) * S + (kt + 1) * 128],
                                    rhs=qkt[:, h * S + qb * QB + qo : h * S + (qb + 1) * QB],
                                    start=True,
                                    stop=True,
                                )
                                p = pp.tile([128, QB], BF16, name="p", tag="p")
                                nc.scalar.activation(
                                    p[:, qo:], psS[:, qo:],
                                    mybir.ActivationFunctionType.Exp,
                                    scale=float(SCALE),
                                )
                                if qo > 0 and kt - kg == 1:
                                    # this tile's denominator stays full
                                    # width (it carries its parity row's
                                    # stop flag) — zero the prefix
                                    nc.vector.memset(p[:, :qo], 0.0)
                                if diag:
                                    # multiplicative 0/1 causal mask on DVE
                                    jj = kt - 4 * qb
                                    nc.vector.tensor_mul(
                                        p[:, qo:], p[:, qo:],
                                        masks_sb[:, jj * QB + qo : (jj + 1) * QB],
                                    )
                                ps_group.append((p, kt, qo))
                            for p, kt, qo in ps_group:
                                if diag and tri_diag:
                                    stop = qo == 0
                                elif is_causal:
                                    stop = gi == ngroups - 1 and kt == kg + 3
                                else:
                                    stop = gi == ngroups - 1 and kt == kg + 3
                                nc.tensor.matmul(
                                    psO[h][:, qo:],
                                    lhsT=vt[:, kt * DL + h * 128 : kt * DL + (h + 1) * 128],
                                    rhs=p[:, qo:],
                                    start=(gi == 0 and kt == kg),
                                    stop=stop,
                                )
                            for p, kt, qo in ps_group:
                                par = (kt - kg) % 2
                                row = 64 * h + 32 * par
                                if diag and tri_diag:
                                    # descending emission: last per parity is
                                    # kt-kg in {0, 1} — those stay full width
                                    # to carry the stop flag; kt-kg in {2, 3}
                                    # only sum their computed span
                                    stop = gi == ngroups - 1 and kt - kg <= 1
                                    dqo = 0 if kt - kg <= 1 else qo
                                else:
                                    stop = gi == ngroups - 1 and kt - kg >= 2
                                    dqo = 0
                                nc.tensor.matmul(
                                    psD[row : row + 1, dqo:],
                                    lhsT=ones_col[:, :],
                                    rhs=p[:, dqo:],
                                    start=(gi == 0 and kt - kg == par),
                                    stop=stop,
                                    tile_position=(0, row),
                                )
                            if gi == ngroups - 1:
                                # emit this head's softmax chain now so it
                                # overlaps the other head's last group
                                # fold the 2 partial-sum rows, then 1/denom
                                dsum = rp.tile([1, QB], F32, name="dsum", tag="dsum")
                                nc.vector.tensor_copy(
                                    dsum[:, :], psD[64 * h : 64 * h + 1, :]
                                )
                                nc.vector.scalar_tensor_tensor(
                                    out=dsum[:, :],
                                    in0=psD[64 * h + 32 : 64 * h + 33, :],
                                    scalar=1.0,
                                    in1=dsum[:, :],
                                    op0=mybir.AluOpType.mult,
                                    op1=mybir.AluOpType.add,
                                )
                                recip = rp.tile([1, QB], F32, name="recip", tag="recip")
                                nc.vector.reciprocal_approx_fast(recip[:, :], dsum[:, :])
                                # broadcast 1/denom across partitions on GpSimd
                                rb = rp.tile([128, QB], F32, name="rb", tag="rb")
                                nc.gpsimd.partition_broadcast(rb[:, :], recip[:, :])
                                o_base = (h * NQB + qb) * QB
                                nc.vector.tensor_mul(
                                    outt[:, o_base : o_base + QB], psO[h][:, :], rb[:, :]
                                )

                    # O-projection for this q-block (both heads ready); per
                    # s-tile, gather the 4 PSUM tiles into one SBUF tile
                    # (copies alternating DVE/ACT) and store with one DMA
                    for j in range(4):
                        st = qb * 4 + j
                        osb = op.tile([128, D], F32, name="osb", tag="osb")
                        for et in range(4):
                            psF = acc4.tile([128, QB], F32, name="psF", tag="acc4")
                            for h in range(NH):
                                o_base = (h * NQB + qb) * QB + j * 128
                                nc.tensor.matmul(
                                    psF[:, :],
                                    lhsT=outt[:, o_base : o_base + 128],
                                    rhs=wot_sb[:, h * D + et * 512 : h * D + (et + 1) * 512],
                                    start=(h == 0),
                                    stop=(h == NH - 1),
                                )
                            if qb == NQB - 1:
                                # tail: ACT is idle — split the drain copy
                                nc.vector.tensor_copy(
                                    osb[:, et * 512 : et * 512 + 256], psF[:, :256]
                                )
                                nc.scalar.copy(
                                    osb[:, et * 512 + 256 : (et + 1) * 512], psF[:, 256:]
                                )
                            else:
                                nc.vector.tensor_copy(
                                    osb[:, et * 512 : (et + 1) * 512], psF[:, :]
                                )
                        nc.sync.dma_start(
                            out=OUT[st * 128 : (st + 1) * 128, :], in_=osb[:, :]
                        )

                if is_causal:
                    # fused schedule: Q for block e first, then slice e's
                    # V/K projection units interleaved INTO attention block
                    # e's k-group stream as PE fill-work
                    for sl in range(NSQ):
                        if sl + 1 < NSQ:
                            load_xe(sl + 1)
                        for dt in range(NH):
                            emit_qk(sl, wq_sb, 0, 0, dt)
                        units = [
                            (lambda s=sl, j=j: emit_v_tile(s, j))
                            for j in range(SQ // 128)
                        ] + [
                            (lambda s=sl, d=d: emit_qk(s, wk_sb, 2, 2, d))
                            for d in range(NH)
                        ]
                        attention_qb(sl, units)
                else:
                    for sl in range(NSQ):
                        if sl + 1 < NSQ:
                            load_xe(sl + 1)
                        proj_slice(sl)
                    for qb in range(NQB):
                        attention_qb(qb)
    nc.finalize()
    return nc


def _bf16(a: np.ndarray) -> np.ndarray:
    return np.ascontiguousarray(a.astype(ml_dtypes.bfloat16))


def make_in_maps(X, Wq, bq, Wk, bk, Wv, bv, Wo, is_causal: bool):
    x2d = np.asarray(X, dtype=np.float32).reshape(S, D)
    # xt2[sl*128+p, et*512+c] = X^T[et*128+p, sl*512+c]
    xt2 = _bf16(
        x2d.T.reshape(NET, 128, NSQ, SQ)
        .transpose(2, 1, 0, 3)
        .reshape(NSQ * 128, NET * SQ)
    )
    masks = np.zeros((128, 4 * QB), dtype=ml_dtypes.bfloat16)
    if is_causal:
        ki = np.arange(128)[:, None]
        qj = np.arange(QB)[None, :]
        for jj in range(4):
            masks[:, jj * QB : (jj + 1) * QB] = (128 * jj + ki <= qj).astype(
                ml_dtypes.bfloat16
            )

    def _pack_w(wT):  # [D, DL] -> [128, NET*DL]
        return _bf16(
            np.ascontiguousarray(wT).reshape(NET, 128, DL)
            .transpose(1, 0, 2)
            .reshape(128, NET * DL)
        )

    in_maps = []
    for c in range(NCORES):
        sl = slice(c * DL, (c + 1) * DL)
        wot = np.asarray(Wo)[:, sl].T  # [DL, D]
        wo2 = _bf16(wot.reshape(NH, 128, D).transpose(1, 0, 2).reshape(128, NH * D))
        in_maps.append(
            {
                "xt2": xt2,
                "wq2": _pack_w(np.asarray(Wq)[sl, :].T),
                "wk2": _pack_w(np.asarray(Wk)[sl, :].T),
                "wv2": _pack_w(np.asarray(Wv)[sl, :].T),
                "bqkc": np.ascontiguousarray(
                    np.stack(
                        [
                            np.asarray(bq, dtype=np.float32)[sl][:128],
                            np.asarray(bq, dtype=np.float32)[sl][128:],
                            np.asarray(bk, dtype=np.float32)[sl][:128],
                            np.asarray(bk, dtype=np.float32)[sl][128:],
                        ],
                        axis=1,
                    )
                ),
                "bvrow": _bf16(np.asarray(bv)[None, sl]),
                "wo2": wo2,
                "masks2": masks,
            }
        )
    return in_maps


_NC_CACHE: dict = {}


def _get_nc(is_causal: bool) -> bass.Bass:
    if is_causal not in _NC_CACHE:
        _NC_CACHE[is_causal] = build_nc(is_causal)
    return _NC_CACHE[is_causal]


def kernel(X, Wq, bq, Wk, bk, Wv, bv, Wo, bo, is_causal, **run_kwargs):
    causal = bool(int(np.asarray(is_causal)))
    nc = _get_nc(causal)
    in_maps = make_in_maps(X, Wq, bq, Wk, bk, Wv, bv, Wo, causal)
    res = run_bass_kernel_spmd(nc, in_maps, core_ids=list(range(NCORES)), **run_kwargs)
    out = np.asarray(bo, dtype=np.float32)[None, :].repeat(S, axis=0)
    for c in range(NCORES):
        out += res.results[c]["out"]
    return out.reshape(1, S, D)


# revision 32
# speedup vs baseline: 1.0054x; 1.0054x over previous
"""Trainium2 Bass kernel for nn_MultiHeadAttention (B=1, S=4096, D=2048, H=16, HD=128).

Sharding: tensor-parallel over heads — 2 heads per core on 8 NeuronCores.
Each core computes its 2 heads' Q/K/V projections, causal attention, and a
partial output projection (row-split Wo); the host sums the 8 partials and
adds the output bias (the all-reduce/unshard step).

Layout strategy (per core, all matmuls bf16 with fp32 PSUM accumulation):
  - X^T [2048, 4096] streamed in eight 512-column slices (double-buffered).
    Projections and attention are FUSED: slice e's Q projection is emitted
    first (it gates block e's scores), then its V/K projection units are
    interleaved INTO attention block e's k-group stream as PE fill-work
    while ACT catches up on the exp queue (K/V land before the diagonal
    group, which needs them). Causal attention for block e only needs K/V
    from slices <= e.
  - All inputs are host-reformatted so every tensor loads with ONE
    contiguous 2D DMA (the sync-engine DMA issue rate, ~0.6us/descriptor,
    was the startup bottleneck with per-tile DMAs). The first X slice is split
    into 8 et-pair transfers interleaved with the V-weight chunks so the
    first V matmuls chase the DMA stream from ~1 MB in.
  - Q, K are produced transposed: QT/KT [d, s]. Scores are computed
    transposed, S^T[k, q] = KT_tile^T @ QT, so that p = exp(S^T) tiles have
    k on partitions -> attn@V needs no transpose.
  - Causal masking is multiplicative (0/1) on DVE after the exp. The
    diagonal k-group of each q-block (qb >= 1) is processed triangularly:
    k-tile 4qb+i only computes q columns >= 128i (emitted descending so the
    final full-width matmul carries the PSUM stop flag). Denominator
    matmuls are narrowed too; only the kt-kg==1 tile stays full width (it
    carries its parity row's stop flag, with a zeroed p prefix).
    (The 512-free matmul stream advances at ~216ns/instr median with a
    stall tail from exp-latency coupling; scores need the deepest PSUM
    ring available — narrower rings or wider 2-bank tiles regress.)
  - Softmax denominators: ones-column matmuls packed 8 rows into ONE
    PSUM bank (head*64 + 32*par); DVE folds each head's two rows (two
    serial ops — DVE cannot read 2 PSUM operands in one instruction),
    reciprocal_approx_fast inverts, GpSimd partition_broadcast spreads
    1/denom for the normalize multiply. Each head's chain is emitted right
    after its last denominator so it overlaps the other head's last group.
  - O-projection: out[s, e] += outT_h[d, s]^T @ WoT_h[d, e], accumulated over
    both local heads; per s-tile the 4 PSUM results are gathered into one
    [128, 2048] SBUF tile and stored with a single DMA (the last q-block's
    drain copies are split across DVE and the by-then-idle ACT engine).

Build notes:
  - Built with bacc.Bacc: walrus encodes at most ONE sem wait per
    instruction, and Bacc's generate_event_semaphores pass splits larger
    wait sets into event-semaphore chains.
  - PSUM: shared [128,512] pool (projections + scores) bufs=5; one
    accumulator ring (psO h0/h1, psD, then the 16 O-proj psF tiles —
    lifetimes are sequential within a q-block) bufs=3 — exactly 8 banks.
"""

import numpy as np
import ml_dtypes

import concourse.bass as bass
import concourse.mybir as mybir
import concourse.tile as tile
from concourse import bacc
from concourse.bass_utils import run_bass_kernel_spmd


S = 4096          # sequence length
D = 2048          # model dim
NCORES = 8
DL = D // NCORES  # 256 local head dims (2 heads)
NH = 2            # heads per core
HD = 128          # head dim
QB = 512          # q block width
NQB = S // QB     # 8
KT = 128          # k tile (partitions)
NKT = S // KT     # 32
ET = 128          # e contraction tile
NET = D // ET     # 16
NST = S // 128    # 32 s-tiles
SQ = 512          # X^T streaming slice width (s columns)
NSQ = S // SQ     # 8 slices
SCALE = 1.0 / np.sqrt(HD)

BF16 = mybir.dt.bfloat16
F32 = mybir.dt.float32


def build_nc(is_causal: bool) -> bass.Bass:
    # Bacc (not raw Bass): its finalize() pipeline splits multi-sem sync
    # waits into event-semaphore chains — walrus encodes at most one wait
    # per instruction.
    nc = bacc.Bacc()

    # xt2 row-block sl: [128, et*512+c] = X[sl*512+c, et*128+p] (host packed)
    XT2 = nc.dram_tensor("xt2", [NSQ * 128, NET * SQ], BF16, kind="ExternalInput")
    # weights packed [128, et*256+c] = W^T[et*128+p, c]
    WQ2 = nc.dram_tensor("wq2", [128, NET * DL], BF16, kind="ExternalInput")
    WK2 = nc.dram_tensor("wk2", [128, NET * DL], BF16, kind="ExternalInput")
    WV2 = nc.dram_tensor("wv2", [128, NET * DL], BF16, kind="ExternalInput")
    # bias columns [128, 4]: bq.d0 | bq.d1 | bk.d0 | bk.d1
    BQKC = nc.dram_tensor("bqkc", [128, 4], F32, kind="ExternalInput")
    BVROW = nc.dram_tensor("bvrow", [1, DL], BF16, kind="ExternalInput")
    # [128, h*2048+c] = Wo^T[h*128+p, c]
    WO2 = nc.dram_tensor("wo2", [128, NH * D], BF16, kind="ExternalInput")
    # [128, jj*512+q]: multiplicative causal masks (1 below/on diagonal)
    MASKS2 = nc.dram_tensor("masks2", [128, 4 * QB], BF16, kind="ExternalInput")
    OUT = nc.dram_tensor("out", [S, D], F32, kind="ExternalOutput")

    with tile.TileContext(nc) as tc:
        with tc.tile_pool(name="persist", bufs=1) as persist:
            # Q head0 | Q head1 | K head0 | K head1, each [128, 4096]
            qkt = persist.tile([128, 4 * S], BF16, name="qkt")
            # V natural layout: s-tile st at cols [st*256, (st+1)*256), head h at +h*128
            vt = persist.tile([128, NST * DL], BF16, name="vt")
            wot_sb = persist.tile([128, NH * D], BF16, name="wot_sb")
            masks_sb = persist.tile([128, 4 * QB], BF16, name="masks_sb")
            wv_sb = persist.tile([128, NET * DL], BF16, name="wv_sb")
            wk_sb = persist.tile([128, NET * DL], BF16, name="wk_sb")
            wq_sb = persist.tile([128, NET * DL], BF16, name="wq_sb")
            ones_col = persist.tile([128, 1], BF16, name="ones_col")
            biasqk = persist.tile([128, 4], F32, name="biasqk")
            bvrow_sb = persist.tile([1, DL], BF16, name="bvrow_sb")
            bvb_sb = persist.tile([128, DL], BF16, name="bvb_sb")
            # normalized attention outputs, transposed: (h*NQB+qb) tile [128d, 512q]
            outt = persist.tile([128, NH * NQB * QB], BF16, name="outt")

            nc.vector.memset(ones_col[:, :], 1.0)

            with tc.tile_pool(name="xtp", bufs=2) as xtp, \
                 tc.tile_pool(name="ps512", bufs=5, space="PSUM") as ps512, \
                 tc.tile_pool(name="acc4", bufs=3, space="PSUM") as acc4, \
                 tc.tile_pool(name="pp", bufs=14) as pp, \
                 tc.tile_pool(name="rp", bufs=4) as rp, \
                 tc.tile_pool(name="op", bufs=3) as op:

                # DMA order: V weights + X^T slice 0, chunked and interleaved
                # so the first V matmuls start after ~1 MB; then the rest.
                xt_tiles = {}
                def load_xe(sl):
                    xt_e = xtp.tile([128, NET * SQ], BF16, name="xt_e", tag="xt")
                    xt_tiles[sl] = xt_e
                    nc.sync.dma_start(
                        out=xt_e[:, :], in_=XT2[sl * 128 : (sl + 1) * 128, :]
                    )
                xt0 = xtp.tile([128, NET * SQ], BF16, name="xt_e", tag="xt")
                xt_tiles[0] = xt0
                WQC = NET * DL // 4   # weight chunk columns
                XPC = NET * SQ // 8   # x-slice et-pair columns
                for ch in range(4):
                    nc.sync.dma_start(
                        out=wv_sb[:, ch * WQC : (ch + 1) * WQC],
                        in_=WV2[:, ch * WQC : (ch + 1) * WQC],
                    )
                    for half in range(2):
                        pc = 2 * ch + half
                        nc.sync.dma_start(
                            out=xt0[:, pc * XPC : (pc + 1) * XPC],
                            in_=XT2[0:128, pc * XPC : (pc + 1) * XPC],
                        )
                nc.sync.dma_start(out=biasqk[:, :], in_=BQKC[:, :])
                nc.sync.dma_start(out=wq_sb[:, :], in_=WQ2[:, :])
                if is_causal:
                    nc.sync.dma_start(out=masks_sb[:, :], in_=MASKS2[:, :])
                nc.sync.dma_start(out=wk_sb[:, :], in_=WK2[:, :])
                nc.sync.dma_start(out=bvrow_sb[:, :], in_=BVROW[:, :])
                nc.sync.dma_start(out=wot_sb[:, :], in_=WO2[:, :])
                # broadcast bv across partitions once; folded into each V
                # tile's PSUM->SBUF copy below
                nc.gpsimd.partition_broadcast(bvb_sb[:, :], bvrow_sb[:, :])

                def emit_v_tile(sl, stl):
                    xt_e = xt_tiles[sl]
                    st = sl * (SQ // 128) + stl
                    psv = ps512.tile([128, QB], F32, name="psv", tag="ps512")
                    for et in range(NET):
                        nc.tensor.matmul(
                            psv[:, :DL],
                            lhsT=xt_e[:, et * SQ + stl * 128 : et * SQ + (stl + 1) * 128],
                            rhs=wv_sb[:, et * DL : (et + 1) * DL],
                            start=(et == 0),
                            stop=(et == NET - 1),
                        )
                    nc.vector.scalar_tensor_tensor(
                        out=vt[:, st * DL : (st + 1) * DL],
                        in0=psv[:, :DL],
                        scalar=1.0,
                        in1=bvb_sb[:, :],
                        op0=mybir.AluOpType.mult,
                        op1=mybir.AluOpType.add,
                    )

                def emit_qk(sl, w_sb, base4, bias_base, dt):
                    # transposed [d, s] projection for one head; bias add on
                    # ACT (per-partition bias) to keep DVE free
                    xt_e = xt_tiles[sl]
                    psq = ps512.tile([128, QB], F32, name="psq", tag="ps512")
                    for et in range(NET):
                        nc.tensor.matmul(
                            psq[:, :],
                            lhsT=w_sb[:, et * DL + dt * 128 : et * DL + (dt + 1) * 128],
                            rhs=xt_e[:, et * SQ : (et + 1) * SQ],
                            start=(et == 0),
                            stop=(et == NET - 1),
                        )
                    nc.scalar.add(
                        qkt[:, (base4 + dt) * S + sl * QB : (base4 + dt) * S + (sl + 1) * QB],
                        psq[:, :],
                        biasqk[:, bias_base + dt : bias_base + dt + 1],
                    )

                def proj_slice(sl):
                    for stl in range(SQ // 128):
                        emit_v_tile(sl, stl)
                    for w_sb, base4, bias_base in ((wq_sb, 0, 0), (wk_sb, 2, 2)):
                        for dt in range(NH):
                            emit_qk(sl, w_sb, base4, bias_base, dt)

                def attention_qb(qb, units=None):
                    kmax = 4 * (qb + 1) if is_causal else NKT
                    groups = list(range(0, kmax, 4))
                    ngroups = len(groups)
                    # triangular diagonal handling needs a preceding
                    # rectangular group to own the psO start flags
                    tri_diag = is_causal and qb >= 1
                    psO = {}
                    for h in range(NH):
                        psO[h] = acc4.tile([128, QB], F32, name="psO", tag="acc4")
                    # one PSUM bank holds all 8 denominator rows:
                    # row = 64*h + 32*par; two k-tiles per row per group
                    psD = acc4.tile([128, QB], F32, name="psD", tag="acc4")
                    units = list(units) if units else []
                    per_gap = -(-len(units) // max(1, ngroups - 1)) if units else 0
                    for gi, kg in enumerate(groups):
                        diag = is_causal and kg == 4 * qb
                        # interleave projection units into the attention
                        # stream: they give PE fill-work while ACT catches up
                        # on the exp queue. K/V of this slice must land
                        # before the diagonal group.
                        if units and diag:
                            while units:
                                units.pop(0)()
                        elif units and gi > 0:
                            for _ in range(per_gap):
                                if units:
                                    units.pop(0)()
                        for h in range(NH):
                            # (kt, q column offset) pairs; descending kt for
                            # the triangular diagonal so the final matmul is
                            # full-width and carries the stop flag
                            if diag and tri_diag:
                                tiles = [(kg + i, i * 128) for i in (3, 2, 1, 0)]
                            else:
                                tiles = [(kg + i, 0) for i in range(4)]
                            ps_group = []
                            for kt, qo in tiles:
                                psS = ps512.tile([128, QB], F32, name="psS", tag="ps512")
                                nc.tensor.matmul(
                                    psS[:, qo:],
                                    lhsT=qkt[:, (2 + h) * S + kt * 128 : (2 + h) * S + (kt + 1) * 128],
                                    rhs=qkt[:, h * S + qb * QB + qo : h * S + (qb + 1) * QB],
                                    start=True,
                                    stop=True,
                                )
                                p = pp.tile([128, QB], BF16, name="p", tag="p")
                                nc.scalar.activation(
                                    p[:, qo:], psS[:, qo:],
                                    mybir.ActivationFunctionType.Exp,
                                    scale=float(SCALE),
                                )
                                if qo > 0 and kt - kg == 1:
                                    # this tile's denominator stays full
                                    # width (it carries its parity row's
                                    # stop flag) — zero the prefix
                                    nc.vector.memset(p[:, :qo], 0.0)
                                if diag:
                                    # multiplicative 0/1 causal mask on DVE
                                    jj = kt - 4 * qb
                                    nc.vector.tensor_mul(
                                        p[:, qo:], p[:, qo:],
                                        masks_sb[:, jj * QB + qo : (jj + 1) * QB],
                                    )
                                ps_group.append((p, kt, qo))
                            for p, kt, qo in ps_group:
                                if diag and tri_diag:
                                    stop = qo == 0
                                elif is_causal:
                                    stop = gi == ngroups - 1 and kt == kg + 3
                                else:
                                    stop = gi == ngroups - 1 and kt == kg + 3
                                nc.tensor.matmul(
                                    psO[h][:, qo:],
                                    lhsT=vt[:, kt * DL + h * 128 : kt * DL + (h + 1) * 128],
                                    rhs=p[:, qo:],
                                    start=(gi == 0 and kt == kg),
                                    stop=stop,
                                )
                            for p, kt, qo in ps_group:
                                par = (kt - kg) % 2
                                row = 64 * h + 32 * par
                                if diag and tri_diag:
                                    # descending emission: last per parity is
                                    # kt-kg in {0, 1} — those stay full width
                                    # to carry the stop flag; kt-kg in {2, 3}
                                    # only sum their computed span
                                    stop = gi == ngroups - 1 and kt - kg <= 1
                                    dqo = 0 if kt - kg <= 1 else qo
                                else:
                                    stop = gi == ngroups - 1 and kt - kg >= 2
                                    dqo = 0
                                nc.tensor.matmul(
                                    psD[row : row + 1, dqo:],
                                    lhsT=ones_col[:, :],
                                    rhs=p[:, dqo:],
                                    start=(gi == 0 and kt - kg == par),
                                    stop=stop,
                                    tile_position=(0, row),
                                )
                            if gi == ngroups - 1:
                                # emit this head's softmax chain now so it
                                # overlaps the other head's last group
                                # fold the 2 partial-sum rows, then 1/denom
                                dsum = rp.tile([1, QB], F32, name="dsum", tag="dsum")
                                nc.vector.tensor_copy(
                                    dsum[:, :], psD[64 * h : 64 * h + 1, :]
                                )
                                nc.vector.scalar_tensor_tensor(
                                    out=dsum[:, :],
                                    in0=psD[64 * h + 32 : 64 * h + 33, :],
                                    scalar=1.0,
                                    in1=dsum[:, :],
                                    op0=mybir.AluOpType.mult,
                                    op1=mybir.AluOpType.add,
                                )
                                recip = rp.tile([1, QB], F32, name="recip", tag="recip")
                                nc.vector.reciprocal_approx_fast(recip[:, :], dsum[:, :])
                                # broadcast 1/denom across partitions on GpSimd
                                rb = rp.tile([128, QB], F32, name="rb", tag="rb")
                                nc.gpsimd.partition_broadcast(rb[:, :], recip[:, :])
                                o_base = (h * NQB + qb) * QB
                                nc.vector.tensor_mul(
                                    outt[:, o_base : o_base + QB], psO[h][:, :], rb[:, :]
                                )

                    # O-projection for this q-block (both heads ready); per
                    # s-tile, gather the 4 PSUM tiles into one SBUF tile
                    # (copies alternating DVE/ACT) and store with one DMA
                    for j in range(4):
                        st = qb * 4 + j
                        osb = op.tile([128, D], F32, name="osb", tag="osb")
                        for et in range(4):
                            psF = acc4.tile([128, QB], F32, name="psF", tag="acc4")
                            for h in range(NH):
                                o_base = (h * NQB + qb) * QB + j * 128
                                nc.tensor.matmul(
                                    psF[:, :],
                                    lhsT=outt[:, o_base : o_base + 128],
                                    rhs=wot_sb[:, h * D + et * 512 : h * D + (et + 1) * 512],
                                    start=(h == 0),
                                    stop=(h == NH - 1),
                                )
                            if qb == NQB - 1:
                                # tail: ACT is idle — split the drain copy
                                nc.vector.tensor_copy(
                                    osb[:, et * 512 : et * 512 + 256], psF[:, :256]
                                )
                                nc.scalar.copy(
                                    osb[:, et * 512 + 256 : (et + 1) * 512], psF[:, 256:]
                                )
                            else:
                                nc.vector.tensor_copy(
                                    osb[:, et * 512 : (et + 1) * 512], psF[:, :]
                                )
                        nc.sync.dma_start(
                            out=OUT[st * 128 : (st + 1) * 128, :], in_=osb[:, :]
                        )

                if is_causal:
                    # fused schedule: Q for block e first, then slice e's
                    # V/K projection units interleaved INTO attention block
                    # e's k-group stream as PE fill-work. Slice 0 instead
                    # runs V first — the V matmuls chase the chunked
                    # wv/x0 DMA stream from ~1 MB in, while Q's wq DMA only
                    # lands after the whole startup burst.
                    for sl in range(NSQ):
                        if sl + 1 < NSQ:
                            load_xe(sl + 1)
                        if sl == 0:
                            for j in range(SQ // 128):
                                emit_v_tile(0, j)
                            for d in range(NH):
                                emit_qk(0, wk_sb, 2, 2, d)
                            for d in range(NH):
                                emit_qk(0, wq_sb, 0, 0, d)
                            attention_qb(0, [])
                            continue
                        for dt in range(NH):
                            emit_qk(sl, wq_sb, 0, 0, dt)
                        units = [
                            (lambda s=sl, j=j: emit_v_tile(s, j))
                            for j in range(SQ // 128)
                        ] + [
                            (lambda s=sl, d=d: emit_qk(s, wk_sb, 2, 2, d))
                            for d in range(NH)
                        ]
                        attention_qb(sl, units)
                else:
                    for sl in range(NSQ):
                        if sl + 1 < NSQ:
                            load_xe(sl + 1)
                        proj_slice(sl)
                    for qb in range(NQB):
                        attention_qb(qb)
    nc.finalize()
    return nc


def _bf16(a: np.ndarray) -> np.ndarray:
    return np.ascontiguousarray(a.astype(ml_dtypes.bfloat16))


def make_in_maps(X, Wq, bq, Wk, bk, Wv, bv, Wo, is_causal: bool):
    x2d = np.asarray(X, dtype=np.float32).reshape(S, D)
    # xt2[sl*128+p, et*512+c] = X^T[et*128+p, sl*512+c]
    xt2 = _bf16(
        x2d.T.reshape(NET, 128, NSQ, SQ)
        .transpose(2, 1, 0, 3)
        .reshape(NSQ * 128, NET * SQ)
    )
    masks = np.zeros((128, 4 * QB), dtype=ml_dtypes.bfloat16)
    if is_causal:
        ki = np.arange(128)[:, None]
        qj = np.arange(QB)[None, :]
        for jj in range(4):
            masks[:, jj * QB : (jj + 1) * QB] = (128 * jj + ki <= qj).astype(
                ml_dtypes.bfloat16
            )

    def _pack_w(wT):  # [D, DL] -> [128, NET*DL]
        return _bf16(
            np.ascontiguousarray(wT).reshape(NET, 128, DL)
            .transpose(1, 0, 2)
            .reshape(128, NET * DL)
        )

    in_maps = []
    for c in range(NCORES):
        sl = slice(c * DL, (c + 1) * DL)
        wot = np.asarray(Wo)[:, sl].T  # [DL, D]
        wo2 = _bf16(wot.reshape(NH, 128, D).transpose(1, 0, 2).reshape(128, NH * D))
        in_maps.append(
            {
                "xt2": xt2,
                "wq2": _pack_w(np.asarray(Wq)[sl, :].T),
                "wk2": _pack_w(np.asarray(Wk)[sl, :].T),
                "wv2": _pack_w(np.asarray(Wv)[sl, :].T),
                "bqkc": np.ascontiguousarray(
                    np.stack(
                        [
                            np.asarray(bq, dtype=np.float32)[sl][:128],
                            np.asarray(bq, dtype=np.float32)[sl][128:],
                            np.asarray(bk, dtype=np.float32)[sl][:128],
                            np.asarray(bk, dtype=np.float32)[sl][128:],
                        ],
                        axis=1,
                    )
                ),
                "bvrow": _bf16(np.asarray(bv)[None, sl]),
                "wo2": wo2,
                "masks2": masks,
            }
        )
    return in_maps


_NC_CACHE: dict = {}


def _get_nc(is_causal: bool) -> bass.Bass:
    if is_causal not in _NC_CACHE:
        _NC_CACHE[is_causal] = build_nc(is_causal)
    return _NC_CACHE[is_causal]


def kernel(X, Wq, bq, Wk, bk, Wv, bv, Wo, bo, is_causal, **run_kwargs):
    causal = bool(int(np.asarray(is_causal)))
    nc = _get_nc(causal)
    in_maps = make_in_maps(X, Wq, bq, Wk, bk, Wv, bv, Wo, causal)
    res = run_bass_kernel_spmd(nc, in_maps, core_ids=list(range(NCORES)), **run_kwargs)
    out = np.asarray(bo, dtype=np.float32)[None, :].repeat(S, axis=0)
    for c in range(NCORES):
        out += res.results[c]["out"]
    return out.reshape(1, S, D)


# revision 33
# speedup vs baseline: 1.0074x; 1.0020x over previous
"""Trainium2 Bass kernel for nn_MultiHeadAttention (B=1, S=4096, D=2048, H=16, HD=128).

Sharding: tensor-parallel over heads — 2 heads per core on 8 NeuronCores.
Each core computes its 2 heads' Q/K/V projections, causal attention, and a
partial output projection (row-split Wo); the host sums the 8 partials and
adds the output bias (the all-reduce/unshard step).

Layout strategy (per core, all matmuls bf16 with fp32 PSUM accumulation):
  - X^T [2048, 4096] streamed in eight 512-column slices (double-buffered).
    Projections and attention are FUSED: slice e's Q projection is emitted
    first (it gates block e's scores), then its V/K projection units are
    interleaved INTO attention block e's k-group stream as PE fill-work
    while ACT catches up on the exp queue (K/V land before the diagonal
    group, which needs them). Causal attention for block e only needs K/V
    from slices <= e.
  - All inputs are host-reformatted so every tensor loads with ONE
    contiguous 2D DMA (the sync-engine DMA issue rate, ~0.6us/descriptor,
    was the startup bottleneck with per-tile DMAs). The first X slice is split
    into 8 et-pair transfers interleaved with the V-weight chunks so the
    first V matmuls chase the DMA stream from ~1 MB in.
  - Q, K are produced transposed: QT/KT [d, s]. Scores are computed
    transposed, S^T[k, q] = KT_tile^T @ QT, so that p = exp(S^T) tiles have
    k on partitions -> attn@V needs no transpose.
  - Causal masking is multiplicative (0/1) on DVE after the exp. The
    diagonal k-group of each q-block (qb >= 1) is processed triangularly:
    k-tile 4qb+i only computes q columns >= 128i (emitted descending so the
    final full-width matmul carries the PSUM stop flag). Denominator
    matmuls are narrowed too; only the kt-kg==1 tile stays full width (it
    carries its parity row's stop flag, with a zeroed p prefix).
    (The 512-free matmul stream advances at ~216ns/instr median with a
    stall tail from exp-latency coupling; scores need the deepest PSUM
    ring available — narrower rings or wider 2-bank tiles regress.)
  - Softmax denominators: ones-column matmuls packed 8 rows into ONE
    PSUM bank (head*64 + 32*par); DVE folds each head's two rows (two
    serial ops — DVE cannot read 2 PSUM operands in one instruction),
    reciprocal_approx_fast inverts, GpSimd partition_broadcast spreads
    1/denom for the normalize multiply. Each head's chain is emitted right
    after its last denominator so it overlaps the other head's last group.
  - O-projection: out[s, e] += outT_h[d, s]^T @ WoT_h[d, e], accumulated over
    both local heads; per s-tile the 4 PSUM results are gathered into one
    [128, 2048] SBUF tile and stored with a single DMA (the last q-block's
    drain copies are split across DVE and the by-then-idle ACT engine).

Build notes:
  - Built with bacc.Bacc: walrus encodes at most ONE sem wait per
    instruction, and Bacc's generate_event_semaphores pass splits larger
    wait sets into event-semaphore chains.
  - PSUM: shared [128,512] pool (projections + scores) bufs=5; one
    accumulator ring (psO h0/h1, psD, then the 16 O-proj psF tiles —
    lifetimes are sequential within a q-block) bufs=3 — exactly 8 banks.
"""

import numpy as np
import ml_dtypes

import concourse.bass as bass
import concourse.mybir as mybir
import concourse.tile as tile
from concourse import bacc
from concourse.bass_utils import run_bass_kernel_spmd


S = 4096          # sequence length
D = 2048          # model dim
NCORES = 8
DL = D // NCORES  # 256 local head dims (2 heads)
NH = 2            # heads per core
HD = 128          # head dim
QB = 512          # q block width
NQB = S // QB     # 8
KT = 128          # k tile (partitions)
NKT = S // KT     # 32
ET = 128          # e contraction tile
NET = D // ET     # 16
NST = S // 128    # 32 s-tiles
SQ = 512          # X^T streaming slice width (s columns)
NSQ = S // SQ     # 8 slices
SCALE = 1.0 / np.sqrt(HD)

BF16 = mybir.dt.bfloat16
F32 = mybir.dt.float32


def build_nc(is_causal: bool) -> bass.Bass:
    # Bacc (not raw Bass): its finalize() pipeline splits multi-sem sync
    # waits into event-semaphore chains — walrus encodes at most one wait
    # per instruction.
    nc = bacc.Bacc()

    # xt2 row-block sl: [128, et*512+c] = X[sl*512+c, et*128+p] (host packed)
    XT2 = nc.dram_tensor("xt2", [NSQ * 128, NET * SQ], BF16, kind="ExternalInput")
    # weights packed [128, et*256+c] = W^T[et*128+p, c]
    WQ2 = nc.dram_tensor("wq2", [128, NET * DL], BF16, kind="ExternalInput")
    WK2 = nc.dram_tensor("wk2", [128, NET * DL], BF16, kind="ExternalInput")
    WV2 = nc.dram_tensor("wv2", [128, NET * DL], BF16, kind="ExternalInput")
    # bias columns [128, 4]: bq.d0 | bq.d1 | bk.d0 | bk.d1
    BQKC = nc.dram_tensor("bqkc", [128, 4], F32, kind="ExternalInput")
    BVROW = nc.dram_tensor("bvrow", [1, DL], BF16, kind="ExternalInput")
    # [128, h*2048+c] = Wo^T[h*128+p, c]
    WO2 = nc.dram_tensor("wo2", [128, NH * D], BF16, kind="ExternalInput")
    # [128, jj*512+q]: multiplicative causal masks (1 below/on diagonal)
    MASKS2 = nc.dram_tensor("masks2", [128, 4 * QB], BF16, kind="ExternalInput")
    OUT = nc.dram_tensor("out", [S, D], F32, kind="ExternalOutput")

    with tile.TileContext(nc) as tc:
        with tc.tile_pool(name="persist", bufs=1) as persist:
            # Q head0 | Q head1 | K head0 | K head1, each [128, 4096]
            qkt = persist.tile([128, 4 * S], BF16, name="qkt")
            # V natural layout: s-tile st at cols [st*256, (st+1)*256), head h at +h*128
            vt = persist.tile([128, NST * DL], BF16, name="vt")
            wot_sb = persist.tile([128, NH * D], BF16, name="wot_sb")
            masks_sb = persist.tile([128, 4 * QB], BF16, name="masks_sb")
            wv_sbs = [
                persist.tile([128, 4 * DL], BF16, name=f"wv_sb{c}") for c in range(4)
            ]
            wk_sb = persist.tile([128, NET * DL], BF16, name="wk_sb")
            wq_sb = persist.tile([128, NET * DL], BF16, name="wq_sb")
            ones_col = persist.tile([128, 1], BF16, name="ones_col")
            biasqk = persist.tile([128, 4], F32, name="biasqk")
            bvrow_sb = persist.tile([1, DL], BF16, name="bvrow_sb")
            bvb_sb = persist.tile([128, DL], BF16, name="bvb_sb")
            # normalized attention outputs, transposed: (h*NQB+qb) tile [128d, 512q]
            outt = persist.tile([128, NH * NQB * QB], BF16, name="outt")

            nc.vector.memset(ones_col[:, :], 1.0)

            with tc.tile_pool(name="xtp", bufs=2) as xtp, \
                 tc.tile_pool(name="ps512", bufs=5, space="PSUM") as ps512, \
                 tc.tile_pool(name="acc4", bufs=3, space="PSUM") as acc4, \
                 tc.tile_pool(name="pp", bufs=14) as pp, \
                 tc.tile_pool(name="rp", bufs=4) as rp, \
                 tc.tile_pool(name="op", bufs=2) as op:

                # DMA order: V weights + X^T slice 0, chunked and interleaved
                # so the first V matmuls start after ~1 MB; then the rest.
                xt_tiles = {}
                def load_xe(sl):
                    xt_e = xtp.tile([128, NET * SQ], BF16, name="xt_e", tag="xt")
                    xt_tiles[sl] = xt_e
                    nc.sync.dma_start(
                        out=xt_e[:, :], in_=XT2[sl * 128 : (sl + 1) * 128, :]
                    )
                # slice 0: X and V weights live in 4 separate chunk tiles
                # each fed by ONE DMA — reader dependencies are tile-granular,
                # so this is what lets the first V matmuls chase the stream
                xt0c = [
                    persist.tile([128, 4 * SQ], BF16, name=f"xt0c{c}")
                    for c in range(4)
                ]
                XCC = 4 * SQ          # x chunk columns (4 et tiles)
                for ch in range(4):
                    nc.sync.dma_start(
                        out=wv_sbs[ch][:, :], in_=WV2[:, ch * 4 * DL : (ch + 1) * 4 * DL]
                    )
                    nc.sync.dma_start(
                        out=xt0c[ch][:, :], in_=XT2[0:128, ch * XCC : (ch + 1) * XCC]
                    )
                nc.sync.dma_start(out=biasqk[:, :], in_=BQKC[:, :])
                nc.sync.dma_start(out=wq_sb[:, :], in_=WQ2[:, :])
                if is_causal:
                    nc.sync.dma_start(out=masks_sb[:, :], in_=MASKS2[:, :])
                nc.sync.dma_start(out=wk_sb[:, :], in_=WK2[:, :])
                nc.sync.dma_start(out=bvrow_sb[:, :], in_=BVROW[:, :])
                nc.sync.dma_start(out=wot_sb[:, :], in_=WO2[:, :])
                # broadcast bv across partitions once; folded into each V
                # tile's PSUM->SBUF copy below
                nc.gpsimd.partition_broadcast(bvb_sb[:, :], bvrow_sb[:, :])

                def emit_v_tile0(stl):
                    st = stl
                    psv = ps512.tile([128, QB], F32, name="psv", tag="ps512")
                    for et in range(NET):
                        nc.tensor.matmul(
                            psv[:, :DL],
                            lhsT=xt0c[et // 4][:, (et % 4) * SQ + stl * 128 : (et % 4) * SQ + (stl + 1) * 128],
                            rhs=wv_sbs[et // 4][:, (et % 4) * DL : (et % 4 + 1) * DL],
                            start=(et == 0),
                            stop=(et == NET - 1),
                        )
                    nc.vector.scalar_tensor_tensor(
                        out=vt[:, st * DL : (st + 1) * DL],
                        in0=psv[:, :DL],
                        scalar=1.0,
                        in1=bvb_sb[:, :],
                        op0=mybir.AluOpType.mult,
                        op1=mybir.AluOpType.add,
                    )

                def emit_qk0(w_sb, base4, bias_base, dt):
                    psq = ps512.tile([128, QB], F32, name="psq", tag="ps512")
                    for et in range(NET):
                        nc.tensor.matmul(
                            psq[:, :],
                            lhsT=w_sb[:, et * DL + dt * 128 : et * DL + (dt + 1) * 128],
                            rhs=xt0c[et // 4][:, (et % 4) * SQ : (et % 4 + 1) * SQ],
                            start=(et == 0),
                            stop=(et == NET - 1),
                        )
                    nc.scalar.add(
                        qkt[:, (base4 + dt) * S : (base4 + dt) * S + QB],
                        psq[:, :],
                        biasqk[:, bias_base + dt : bias_base + dt + 1],
                    )

                def emit_v_tile(sl, stl):
                    xt_e = xt_tiles[sl]
                    st = sl * (SQ // 128) + stl
                    psv = ps512.tile([128, QB], F32, name="psv", tag="ps512")
                    for et in range(NET):
                        nc.tensor.matmul(
                            psv[:, :DL],
                            lhsT=xt_e[:, et * SQ + stl * 128 : et * SQ + (stl + 1) * 128],
                            rhs=wv_sbs[et // 4][:, (et % 4) * DL : (et % 4 + 1) * DL],
                            start=(et == 0),
                            stop=(et == NET - 1),
                        )
                    nc.vector.scalar_tensor_tensor(
                        out=vt[:, st * DL : (st + 1) * DL],
                        in0=psv[:, :DL],
                        scalar=1.0,
                        in1=bvb_sb[:, :],
                        op0=mybir.AluOpType.mult,
                        op1=mybir.AluOpType.add,
                    )

                def emit_qk(sl, w_sb, base4, bias_base, dt):
                    # transposed [d, s] projection for one head; bias add on
                    # ACT (per-partition bias) to keep DVE free
                    xt_e = xt_tiles[sl]
                    psq = ps512.tile([128, QB], F32, name="psq", tag="ps512")
                    for et in range(NET):
                        nc.tensor.matmul(
                            psq[:, :],
                            lhsT=w_sb[:, et * DL + dt * 128 : et * DL + (dt + 1) * 128],
                            rhs=xt_e[:, et * SQ : (et + 1) * SQ],
                            start=(et == 0),
                            stop=(et == NET - 1),
                        )
                    nc.scalar.add(
                        qkt[:, (base4 + dt) * S + sl * QB : (base4 + dt) * S + (sl + 1) * QB],
                        psq[:, :],
                        biasqk[:, bias_base + dt : bias_base + dt + 1],
                    )

                def proj_slice(sl):
                    for stl in range(SQ // 128):
                        if sl == 0:
                            emit_v_tile0(stl)
                        else:
                            emit_v_tile(sl, stl)
                    for w_sb, base4, bias_base in ((wq_sb, 0, 0), (wk_sb, 2, 2)):
                        for dt in range(NH):
                            if sl == 0:
                                emit_qk0(w_sb, base4, bias_base, dt)
                            else:
                                emit_qk(sl, w_sb, base4, bias_base, dt)

                def attention_qb(qb, units=None):
                    kmax = 4 * (qb + 1) if is_causal else NKT
                    groups = list(range(0, kmax, 4))
                    ngroups = len(groups)
                    # triangular diagonal handling needs a preceding
                    # rectangular group to own the psO start flags
                    tri_diag = is_causal and qb >= 1
                    psO = {}
                    for h in range(NH):
                        psO[h] = acc4.tile([128, QB], F32, name="psO", tag="acc4")
                    # one PSUM bank holds all 8 denominator rows:
                    # row = 64*h + 32*par; two k-tiles per row per group
                    psD = acc4.tile([128, QB], F32, name="psD", tag="acc4")
                    units = list(units) if units else []
                    per_gap = -(-len(units) // max(1, ngroups - 1)) if units else 0
                    for gi, kg in enumerate(groups):
                        diag = is_causal and kg == 4 * qb
                        # interleave projection units into the attention
                        # stream: they give PE fill-work while ACT catches up
                        # on the exp queue. K/V of this slice must land
                        # before the diagonal group.
                        if units and diag:
                            while units:
                                units.pop(0)()
                        elif units and gi > 0:
                            for _ in range(per_gap):
                                if units:
                                    units.pop(0)()
                        for h in range(NH):
                            # (kt, q column offset) pairs; descending kt for
                            # the triangular diagonal so the final matmul is
                            # full-width and carries the stop flag
                            if diag and tri_diag:
                                tiles = [(kg + i, i * 128) for i in (3, 2, 1, 0)]
                            else:
                                tiles = [(kg + i, 0) for i in range(4)]
                            ps_group = []
                            for kt, qo in tiles:
                                psS = ps512.tile([128, QB], F32, name="psS", tag="ps512")
                                nc.tensor.matmul(
                                    psS[:, qo:],
                                    lhsT=qkt[:, (2 + h) * S + kt * 128 : (2 + h) * S + (kt + 1) * 128],
                                    rhs=qkt[:, h * S + qb * QB + qo : h * S + (qb + 1) * QB],
                                    start=True,
                                    stop=True,
                                )
                                p = pp.tile([128, QB], BF16, name="p", tag="p")
                                nc.scalar.activation(
                                    p[:, qo:], psS[:, qo:],
                                    mybir.ActivationFunctionType.Exp,
                                    scale=float(SCALE),
                                )
                                if qo > 0 and kt - kg == 1:
                                    # this tile's denominator stays full
                                    # width (it carries its parity row's
                                    # stop flag) — zero the prefix
                                    nc.vector.memset(p[:, :qo], 0.0)
                                if diag:
                                    # multiplicative 0/1 causal mask on DVE
                                    jj = kt - 4 * qb
                                    nc.vector.tensor_mul(
                                        p[:, qo:], p[:, qo:],
                                        masks_sb[:, jj * QB + qo : (jj + 1) * QB],
                                    )
                                ps_group.append((p, kt, qo))
                            for p, kt, qo in ps_group:
                                if diag and tri_diag:
                                    stop = qo == 0
                                elif is_causal:
                                    stop = gi == ngroups - 1 and kt == kg + 3
                                else:
                                    stop = gi == ngroups - 1 and kt == kg + 3
                                nc.tensor.matmul(
                                    psO[h][:, qo:],
                                    lhsT=vt[:, kt * DL + h * 128 : kt * DL + (h + 1) * 128],
                                    rhs=p[:, qo:],
                                    start=(gi == 0 and kt == kg),
                                    stop=stop,
                                )
                            for p, kt, qo in ps_group:
                                par = (kt - kg) % 2
                                row = 64 * h + 32 * par
                                if diag and tri_diag:
                                    # descending emission: last per parity is
                                    # kt-kg in {0, 1} — those stay full width
                                    # to carry the stop flag; kt-kg in {2, 3}
                                    # only sum their computed span
                                    stop = gi == ngroups - 1 and kt - kg <= 1
                                    dqo = 0 if kt - kg <= 1 else qo
                                else:
                                    stop = gi == ngroups - 1 and kt - kg >= 2
                                    dqo = 0
                                nc.tensor.matmul(
                                    psD[row : row + 1, dqo:],
                                    lhsT=ones_col[:, :],
                                    rhs=p[:, dqo:],
                                    start=(gi == 0 and kt - kg == par),
                                    stop=stop,
                                    tile_position=(0, row),
                                )
                            if gi == ngroups - 1:
                                # emit this head's softmax chain now so it
                                # overlaps the other head's last group
                                # fold the 2 partial-sum rows, then 1/denom
                                dsum = rp.tile([1, QB], F32, name="dsum", tag="dsum")
                                nc.vector.tensor_copy(
                                    dsum[:, :], psD[64 * h : 64 * h + 1, :]
                                )
                                nc.vector.scalar_tensor_tensor(
                                    out=dsum[:, :],
                                    in0=psD[64 * h + 32 : 64 * h + 33, :],
                                    scalar=1.0,
                                    in1=dsum[:, :],
                                    op0=mybir.AluOpType.mult,
                                    op1=mybir.AluOpType.add,
                                )
                                recip = rp.tile([1, QB], F32, name="recip", tag="recip")
                                nc.vector.reciprocal_approx_fast(recip[:, :], dsum[:, :])
                                # broadcast 1/denom across partitions on GpSimd
                                rb = rp.tile([128, QB], F32, name="rb", tag="rb")
                                nc.gpsimd.partition_broadcast(rb[:, :], recip[:, :])
                                o_base = (h * NQB + qb) * QB
                                nc.vector.tensor_mul(
                                    outt[:, o_base : o_base + QB], psO[h][:, :], rb[:, :]
                                )

                    # O-projection for this q-block (both heads ready); per
                    # s-tile, gather the 4 PSUM tiles into one SBUF tile
                    # (copies alternating DVE/ACT) and store with one DMA
                    for j in range(4):
                        st = qb * 4 + j
                        osb = op.tile([128, D], F32, name="osb", tag="osb")
                        for et in range(4):
                            psF = acc4.tile([128, QB], F32, name="psF", tag="acc4")
                            for h in range(NH):
                                o_base = (h * NQB + qb) * QB + j * 128
                                nc.tensor.matmul(
                                    psF[:, :],
                                    lhsT=outt[:, o_base : o_base + 128],
                                    rhs=wot_sb[:, h * D + et * 512 : h * D + (et + 1) * 512],
                                    start=(h == 0),
                                    stop=(h == NH - 1),
                                )
                            if qb == NQB - 1:
                                # tail: ACT is idle — split the drain copy
                                nc.vector.tensor_copy(
                                    osb[:, et * 512 : et * 512 + 256], psF[:, :256]
                                )
                                nc.scalar.copy(
                                    osb[:, et * 512 + 256 : (et + 1) * 512], psF[:, 256:]
                                )
                            else:
                                nc.vector.tensor_copy(
                                    osb[:, et * 512 : (et + 1) * 512], psF[:, :]
                                )
                        nc.sync.dma_start(
                            out=OUT[st * 128 : (st + 1) * 128, :], in_=osb[:, :]
                        )

                if is_causal:
                    # fused schedule: Q for block e first, then slice e's
                    # V/K projection units interleaved INTO attention block
                    # e's k-group stream as PE fill-work. Slice 0 instead
                    # runs V first — the V matmuls chase the chunked
                    # wv/x0 DMA stream from ~1 MB in, while Q's wq DMA only
                    # lands after the whole startup burst.
                    for sl in range(NSQ):
                        if sl + 1 < NSQ:
                            load_xe(sl + 1)
                        if sl == 0:
                            for j in range(SQ // 128):
                                emit_v_tile0(j)
                            for d in range(NH):
                                emit_qk0(wk_sb, 2, 2, d)
                            for d in range(NH):
                                emit_qk0(wq_sb, 0, 0, d)
                            attention_qb(0, [])
                            continue
                        for dt in range(NH):
                            emit_qk(sl, wq_sb, 0, 0, dt)
                        units = [
                            (lambda s=sl, j=j: emit_v_tile(s, j))
                            for j in range(SQ // 128)
                        ] + [
                            (lambda s=sl, d=d: emit_qk(s, wk_sb, 2, 2, d))
                            for d in range(NH)
                        ]
                        attention_qb(sl, units)
                else:
                    for sl in range(NSQ):
                        if sl + 1 < NSQ:
                            load_xe(sl + 1)
                        proj_slice(sl)
                    for qb in range(NQB):
                        attention_qb(qb)
    nc.finalize()
    return nc


def _bf16(a: np.ndarray) -> np.ndarray:
    return np.ascontiguousarray(a.astype(ml_dtypes.bfloat16))


def make_in_maps(X, Wq, bq, Wk, bk, Wv, bv, Wo, is_causal: bool):
    x2d = np.asarray(X, dtype=np.float32).reshape(S, D)
    # xt2[sl*128+p, et*512+c] = X^T[et*128+p, sl*512+c]
    xt2 = _bf16(
        x2d.T.reshape(NET, 128, NSQ, SQ)
        .transpose(2, 1, 0, 3)
        .reshape(NSQ * 128, NET * SQ)
    )
    masks = np.zeros((128, 4 * QB), dtype=ml_dtypes.bfloat16)
    if is_causal:
        ki = np.arange(128)[:, None]
        qj = np.arange(QB)[None, :]
        for jj in range(4):
            masks[:, jj * QB : (jj + 1) * QB] = (128 * jj + ki <= qj).astype(
                ml_dtypes.bfloat16
            )

    def _pack_w(wT):  # [D, DL] -> [128, NET*DL]
        return _bf16(
            np.ascontiguousarray(wT).reshape(NET, 128, DL)
            .transpose(1, 0, 2)
            .reshape(128, NET * DL)
        )

    in_maps = []
    for c in range(NCORES):
        sl = slice(c * DL, (c + 1) * DL)
        wot = np.asarray(Wo)[:, sl].T  # [DL, D]
        wo2 = _bf16(wot.reshape(NH, 128, D).transpose(1, 0, 2).reshape(128, NH * D))
        in_maps.append(
            {
                "xt2": xt2,
                "wq2": _pack_w(np.asarray(Wq)[sl, :].T),
                "wk2": _pack_w(np.asarray(Wk)[sl, :].T),
                "wv2": _pack_w(np.asarray(Wv)[sl, :].T),
                "bqkc": np.ascontiguousarray(
                    np.stack(
                        [
                            np.asarray(bq, dtype=np.float32)[sl][:128],
                            np.asarray(bq, dtype=np.float32)[sl][128:],
                            np.asarray(bk, dtype=np.float32)[sl][:128],
                            np.asarray(bk, dtype=np.float32)[sl][128:],
                        ],
                        axis=1,
                    )
                ),
                "bvrow": _bf16(np.asarray(bv)[None, sl]),
                "wo2": wo2,
                "masks2": masks,
            }
        )
    return in_maps


_NC_CACHE: dict = {}


def _get_nc(is_causal: bool) -> bass.Bass:
    if is_causal not in _NC_CACHE:
        _NC_CACHE[is_causal] = build_nc(is_causal)
    return _NC_CACHE[is_causal]


def kernel(X, Wq, bq, Wk, bk, Wv, bv, Wo, bo, is_causal, **run_kwargs):
    causal = bool(int(np.asarray(is_causal)))
    nc = _get_nc(causal)
    in_maps = make_in_maps(X, Wq, bq, Wk, bk, Wv, bv, Wo, causal)
    res = run_bass_kernel_spmd(nc, in_maps, core_ids=list(range(NCORES)), **run_kwargs)
    out = np.asarray(bo, dtype=np.float32)[None, :].repeat(S, axis=0)
    for c in range(NCORES):
        out += res.results[c]["out"]
    return out.reshape(1, S, D)


# revision 35
# speedup vs baseline: 1.0254x; 1.0178x over previous
"""Trainium2 Bass kernel for nn_MultiHeadAttention (B=1, S=4096, D=2048, H=16, HD=128).

Sharding: tensor-parallel over heads — 2 heads per core on 8 NeuronCores.
Each core computes its 2 heads' Q/K/V projections, causal attention, and a
partial output projection (row-split Wo); the host sums the 8 partials and
adds the output bias (the all-reduce/unshard step).

Layout strategy (per core, all matmuls bf16 with fp32 PSUM accumulation):
  - X^T [2048, 4096] streamed in eight 512-column slices (double-buffered).
    Projections and attention are FUSED: slice e's Q projection is emitted
    first (it gates block e's scores), then its V/K projection units are
    interleaved INTO attention block e's k-group stream as PE fill-work
    while ACT catches up on the exp queue (K/V land before the diagonal
    group, which needs them). Causal attention for block e only needs K/V
    from slices <= e.
  - All inputs are host-reformatted so every tensor loads with ONE
    contiguous 2D DMA (the sync-engine DMA issue rate, ~0.6us/descriptor,
    was the startup bottleneck with per-tile DMAs). The first X slice and the
    V weights live in 4 separate chunk TILES (reader dependencies are
    tile-granular, so only separate tiles let the first V matmuls chase
    the DMA stream instead of waiting for the whole startup burst).
  - Q, K are produced transposed: QT/KT [d, s]. Scores are computed
    transposed, S^T[k, q] = KT_tile^T @ QT, so that p = exp(S^T) tiles have
    k on partitions -> attn@V needs no transpose.
  - Causal masking is multiplicative (0/1) on DVE after the exp. The
    diagonal k-group of each q-block (qb >= 1) is processed triangularly:
    k-tile 4qb+i only computes q columns >= 128i (emitted descending so the
    final full-width matmul carries the PSUM stop flag). Denominator
    matmuls are narrowed too; only the kt-kg==1 tile stays full width (it
    carries its parity row's stop flag, with a zeroed p prefix).
    (The 512-free matmul stream advances at ~216ns/instr median with a
    stall tail from exp-latency coupling; scores need the deepest PSUM
    ring available — narrower rings or wider 2-bank tiles regress.)
  - Softmax denominators: ones-column matmuls packed 8 rows into ONE
    PSUM bank (head*64 + 32*par); DVE folds each head's two rows (two
    serial ops — DVE cannot read 2 PSUM operands in one instruction),
    reciprocal_approx_fast inverts, GpSimd partition_broadcast spreads
    1/denom for the normalize multiply. Each head's chain is emitted right
    after its last denominator so it overlaps the other head's last group.
  - O-projection: out[s, e] += outT_h[d, s]^T @ WoT_h[d, e], accumulated over
    both local heads; per s-tile the 4 PSUM results are gathered into one
    [128, 2048] SBUF tile and stored with a single DMA (the last q-block's
    drain copies are split across DVE and the by-then-idle ACT engine).

Build notes:
  - Built with bacc.Bacc: walrus encodes at most ONE sem wait per
    instruction, and Bacc's generate_event_semaphores pass splits larger
    wait sets into event-semaphore chains.
  - PSUM: shared [128,512] pool (projections + scores) bufs=5; one
    accumulator ring (psO h0/h1, psD, then the 16 O-proj psF tiles —
    lifetimes are sequential within a q-block) bufs=3 — exactly 8 banks.
"""

import numpy as np
import ml_dtypes

import concourse.bass as bass
import concourse.mybir as mybir
import concourse.tile as tile
from concourse import bacc
from concourse.bass_utils import run_bass_kernel_spmd


S = 4096          # sequence length
D = 2048          # model dim
NCORES = 8
DL = D // NCORES  # 256 local head dims (2 heads)
NH = 2            # heads per core
HD = 128          # head dim
QB = 512          # q block width
NQB = S // QB     # 8
KT = 128          # k tile (partitions)
NKT = S // KT     # 32
ET = 128          # e contraction tile
NET = D // ET     # 16
NST = S // 128    # 32 s-tiles
SQ = 512          # X^T streaming slice width (s columns)
NSQ = S // SQ     # 8 slices
SCALE = 1.0 / np.sqrt(HD)

BF16 = mybir.dt.bfloat16
F32 = mybir.dt.float32


def build_nc(is_causal: bool) -> bass.Bass:
    # Bacc (not raw Bass): its finalize() pipeline splits multi-sem sync
    # waits into event-semaphore chains — walrus encodes at most one wait
    # per instruction.
    nc = bacc.Bacc()

    # xt2 row-block sl: [128, et*512+c] = X[sl*512+c, et*128+p] (host packed)
    XT2 = nc.dram_tensor("xt2", [NSQ * 128, NET * SQ], BF16, kind="ExternalInput")
    # weights packed [128, et*256+c] = W^T[et*128+p, c]
    WQ2 = nc.dram_tensor("wq2", [128, NET * DL], BF16, kind="ExternalInput")
    WK2 = nc.dram_tensor("wk2", [128, NET * DL], BF16, kind="ExternalInput")
    WV2 = nc.dram_tensor("wv2", [128, NET * DL], BF16, kind="ExternalInput")
    # bias columns [128, 4]: bq.d0 | bq.d1 | bk.d0 | bk.d1
    BQKC = nc.dram_tensor("bqkc", [128, 4], F32, kind="ExternalInput")
    BVROW = nc.dram_tensor("bvrow", [1, DL], BF16, kind="ExternalInput")
    # [128, h*2048+c] = Wo^T[h*128+p, c]
    WO2 = nc.dram_tensor("wo2", [128, NH * D], BF16, kind="ExternalInput")
    # [128, jj*512+q]: multiplicative causal masks (1 below/on diagonal)
    MASKS2 = nc.dram_tensor("masks2", [128, 4 * QB], BF16, kind="ExternalInput")
    OUT = nc.dram_tensor("out", [S, D], F32, kind="ExternalOutput")

    with tile.TileContext(nc) as tc:
        with tc.tile_pool(name="persist", bufs=1) as persist:
            # Q head0 | Q head1 | K head0 | K head1, each [128, 4096]
            qkt = persist.tile([128, 4 * S], BF16, name="qkt")
            # V natural layout: s-tile st at cols [st*256, (st+1)*256), head h at +h*128
            vt = persist.tile([128, NST * DL], BF16, name="vt")
            wot_sb = persist.tile([128, NH * D], BF16, name="wot_sb")
            masks_sb = persist.tile([128, 4 * QB], BF16, name="masks_sb")
            wk_sb = persist.tile([128, NET * DL], BF16, name="wk_sb")
            wq_sb = persist.tile([128, NET * DL], BF16, name="wq_sb")
            ones_col = persist.tile([128, 1], BF16, name="ones_col")
            biasqk = persist.tile([128, 4], F32, name="biasqk")
            bvrow_sb = persist.tile([1, DL], BF16, name="bvrow_sb")
            bvb_sb = persist.tile([128, DL], BF16, name="bvb_sb")
            # normalized attention outputs, transposed: (h*NQB+qb) tile [128d, 512q]
            outt = persist.tile([128, NH * NQB * QB], BF16, name="outt")

            nc.vector.memset(ones_col[:, :], 1.0)

            with tc.tile_pool(name="xtp", bufs=2) as xtp, \
                 tc.tile_pool(name="ps512", bufs=5, space="PSUM") as ps512, \
                 tc.tile_pool(name="acc4", bufs=3, space="PSUM") as acc4, \
                 tc.tile_pool(name="pp", bufs=14) as pp, \
                 tc.tile_pool(name="rp", bufs=4) as rp, \
                 tc.tile_pool(name="op", bufs=2) as op:

                # DMA order: V weights + X^T slice 0, chunked and interleaved
                # so the first V matmuls start after ~1 MB; then the rest.
                xt_tiles = {}
                def load_xe(sl):
                    xt_e = xtp.tile([128, NET * SQ], BF16, name="xt_e", tag="xt")
                    xt_tiles[sl] = xt_e
                    nc.sync.dma_start(
                        out=xt_e[:, :], in_=XT2[sl * 128 : (sl + 1) * 128, :]
                    )
                # slice 0: X and V weights live in separate chunk tiles,
                # each fed by ONE DMA — reader dependencies are tile-granular,
                # so only separate tiles let the first V matmuls chase the
                # stream. Leading chunks are smaller so the first matmul
                # starts earliest.
                XCHUNKS = [2, 2, 4, 4, 4]           # et tiles per chunk
                XOFF = [0, 2, 4, 8, 12]             # et offset per chunk
                def chunk_of(et):
                    for ci in range(len(XCHUNKS) - 1, -1, -1):
                        if et >= XOFF[ci]:
                            return ci, et - XOFF[ci]
                xt0c = [
                    persist.tile([128, n * SQ], BF16, name=f"xt0c{c}")
                    for c, n in enumerate(XCHUNKS)
                ]
                wv_cs = [
                    persist.tile([128, n * DL], BF16, name=f"wv_c{c}")
                    for c, n in enumerate(XCHUNKS)
                ]
                for ci, n in enumerate(XCHUNKS):
                    nc.sync.dma_start(
                        out=wv_cs[ci][:, :],
                        in_=WV2[:, XOFF[ci] * DL : (XOFF[ci] + n) * DL],
                    )
                    nc.sync.dma_start(
                        out=xt0c[ci][:, :],
                        in_=XT2[0:128, XOFF[ci] * SQ : (XOFF[ci] + n) * SQ],
                    )
                nc.sync.dma_start(out=biasqk[:, :], in_=BQKC[:, :])
                nc.sync.dma_start(out=wq_sb[:, :], in_=WQ2[:, :])
                if is_causal:
                    nc.sync.dma_start(out=masks_sb[:, :], in_=MASKS2[:, :])
                nc.sync.dma_start(out=wk_sb[:, :], in_=WK2[:, :])
                nc.sync.dma_start(out=bvrow_sb[:, :], in_=BVROW[:, :])
                nc.sync.dma_start(out=wot_sb[:, :], in_=WO2[:, :])
                # broadcast bv across partitions once; folded into each V
                # tile's PSUM->SBUF copy below
                nc.gpsimd.partition_broadcast(bvb_sb[:, :], bvrow_sb[:, :])

                def emit_v_tile0(stl):
                    st = stl
                    psv = ps512.tile([128, QB], F32, name="psv", tag="ps512")
                    for et in range(NET):
                        ci, le = chunk_of(et)
                        nc.tensor.matmul(
                            psv[:, :DL],
                            lhsT=xt0c[ci][:, le * SQ + stl * 128 : le * SQ + (stl + 1) * 128],
                            rhs=wv_cs[ci][:, le * DL : (le + 1) * DL],
                            start=(et == 0),
                            stop=(et == NET - 1),
                        )
                    nc.vector.scalar_tensor_tensor(
                        out=vt[:, st * DL : (st + 1) * DL],
                        in0=psv[:, :DL],
                        scalar=1.0,
                        in1=bvb_sb[:, :],
                        op0=mybir.AluOpType.mult,
                        op1=mybir.AluOpType.add,
                    )

                def emit_qk0(w_sb, base4, bias_base, dt):
                    psq = ps512.tile([128, QB], F32, name="psq", tag="ps512")
                    for et in range(NET):
                        ci, le = chunk_of(et)
                        nc.tensor.matmul(
                            psq[:, :],
                            lhsT=w_sb[:, et * DL + dt * 128 : et * DL + (dt + 1) * 128],
                            rhs=xt0c[ci][:, le * SQ : (le + 1) * SQ],
                            start=(et == 0),
                            stop=(et == NET - 1),
                        )
                    nc.scalar.add(
                        qkt[:, (base4 + dt) * S : (base4 + dt) * S + QB],
                        psq[:, :],
                        biasqk[:, bias_base + dt : bias_base + dt + 1],
                    )

                def emit_v_tile(sl, stl):
                    xt_e = xt_tiles[sl]
                    st = sl * (SQ // 128) + stl
                    psv = ps512.tile([128, QB], F32, name="psv", tag="ps512")
                    for et in range(NET):
                        nc.tensor.matmul(
                            psv[:, :DL],
                            lhsT=xt_e[:, et * SQ + stl * 128 : et * SQ + (stl + 1) * 128],
                            rhs=wv_cs[chunk_of(et)[0]][:, chunk_of(et)[1] * DL : (chunk_of(et)[1] + 1) * DL],
                            start=(et == 0),
                            stop=(et == NET - 1),
                        )
                    nc.vector.scalar_tensor_tensor(
                        out=vt[:, st * DL : (st + 1) * DL],
                        in0=psv[:, :DL],
                        scalar=1.0,
                        in1=bvb_sb[:, :],
                        op0=mybir.AluOpType.mult,
                        op1=mybir.AluOpType.add,
                    )

                def emit_qk(sl, w_sb, base4, bias_base, dt, on_dve=False):
                    # transposed [d, s] projection for one head. Bias add on
                    # ACT normally; the interleaved K units use DVE so they
                    # do not delay the exp stream queued on ACT.
                    xt_e = xt_tiles[sl]
                    psq = ps512.tile([128, QB], F32, name="psq", tag="ps512")
                    for et in range(NET):
                        nc.tensor.matmul(
                            psq[:, :],
                            lhsT=w_sb[:, et * DL + dt * 128 : et * DL + (dt + 1) * 128],
                            rhs=xt_e[:, et * SQ : (et + 1) * SQ],
                            start=(et == 0),
                            stop=(et == NET - 1),
                        )
                    dst = qkt[:, (base4 + dt) * S + sl * QB : (base4 + dt) * S + (sl + 1) * QB]
                    if on_dve:
                        nc.vector.tensor_scalar_add(
                            out=dst, in0=psq[:, :],
                            scalar1=biasqk[:, bias_base + dt : bias_base + dt + 1],
                        )
                    else:
                        nc.scalar.add(
                            dst, psq[:, :],
                            biasqk[:, bias_base + dt : bias_base + dt + 1],
                        )

                def proj_slice(sl):
                    for stl in range(SQ // 128):
                        if sl == 0:
                            emit_v_tile0(stl)
                        else:
                            emit_v_tile(sl, stl)
                    for w_sb, base4, bias_base in ((wq_sb, 0, 0), (wk_sb, 2, 2)):
                        for dt in range(NH):
                            if sl == 0:
                                emit_qk0(w_sb, base4, bias_base, dt)
                            else:
                                emit_qk(sl, w_sb, base4, bias_base, dt)

                def attention_qb(qb, units=None):
                    kmax = 4 * (qb + 1) if is_causal else NKT
                    groups = list(range(0, kmax, 4))
                    ngroups = len(groups)
                    # triangular diagonal handling needs a preceding
                    # rectangular group to own the psO start flags
                    tri_diag = is_causal and qb >= 1
                    psO = {}
                    for h in range(NH):
                        psO[h] = acc4.tile([128, QB], F32, name="psO", tag="acc4")
                    # one PSUM bank holds all 8 denominator rows:
                    # row = 64*h + 32*par; two k-tiles per row per group
                    psD = acc4.tile([128, QB], F32, name="psD", tag="acc4")
                    units = list(units) if units else []
                    per_gap = -(-len(units) // max(1, ngroups - 1)) if units else 0
                    for gi, kg in enumerate(groups):
                        diag = is_causal and kg == 4 * qb
                        # interleave projection units into the attention
                        # stream: they give PE fill-work while ACT catches up
                        # on the exp queue. K/V of this slice must land
                        # before the diagonal group.
                        if units and diag:
                            while units:
                                units.pop(0)()
                        elif units and gi > 0:
                            for _ in range(per_gap):
                                if units:
                                    units.pop(0)()
                        for h in range(NH):
                            # (kt, q column offset) pairs; descending kt for
                            # the triangular diagonal so the final matmul is
                            # full-width and carries the stop flag
                            if diag and tri_diag:
                                tiles = [(kg + i, i * 128) for i in (3, 2, 1, 0)]
                            else:
                                tiles = [(kg + i, 0) for i in range(4)]
                            ps_group = []
                            for kt, qo in tiles:
                                psS = ps512.tile([128, QB], F32, name="psS", tag="ps512")
                                nc.tensor.matmul(
                                    psS[:, qo:],
                                    lhsT=qkt[:, (2 + h) * S + kt * 128 : (2 + h) * S + (kt + 1) * 128],
                                    rhs=qkt[:, h * S + qb * QB + qo : h * S + (qb + 1) * QB],
                                    start=True,
                                    stop=True,
                                )
                                p = pp.tile([128, QB], BF16, name="p", tag="p")
                                nc.scalar.activation(
                                    p[:, qo:], psS[:, qo:],
                                    mybir.ActivationFunctionType.Exp,
                                    scale=float(SCALE),
                                )
                                if qo > 0 and kt - kg == 1:
                                    # this tile's denominator stays full
                                    # width (it carries its parity row's
                                    # stop flag) — zero the prefix
                                    nc.vector.memset(p[:, :qo], 0.0)
                                if diag:
                                    # multiplicative 0/1 causal mask on DVE
                                    jj = kt - 4 * qb
                                    nc.vector.tensor_mul(
                                        p[:, qo:], p[:, qo:],
                                        masks_sb[:, jj * QB + qo : (jj + 1) * QB],
                                    )
                                ps_group.append((p, kt, qo))
                            for p, kt, qo in ps_group:
                                if diag and tri_diag:
                                    stop = qo == 0
                                elif is_causal:
                                    stop = gi == ngroups - 1 and kt == kg + 3
                                else:
                                    stop = gi == ngroups - 1 and kt == kg + 3
                                nc.tensor.matmul(
                                    psO[h][:, qo:],
                                    lhsT=vt[:, kt * DL + h * 128 : kt * DL + (h + 1) * 128],
                                    rhs=p[:, qo:],
                                    start=(gi == 0 and kt == kg),
                                    stop=stop,
                                )
                            for p, kt, qo in ps_group:
                                par = (kt - kg) % 2
                                row = 64 * h + 32 * par
                                if diag and tri_diag:
                                    # descending emission: last per parity is
                                    # kt-kg in {0, 1} — those stay full width
                                    # to carry the stop flag; kt-kg in {2, 3}
                                    # only sum their computed span
                                    stop = gi == ngroups - 1 and kt - kg <= 1
                                    dqo = 0 if kt - kg <= 1 else qo
                                else:
                                    stop = gi == ngroups - 1 and kt - kg >= 2
                                    dqo = 0
                                nc.tensor.matmul(
                                    psD[row : row + 1, dqo:],
                                    lhsT=ones_col[:, :],
                                    rhs=p[:, dqo:],
                                    start=(gi == 0 and kt - kg == par),
                                    stop=stop,
                                    tile_position=(0, row),
                                )
                            if gi == ngroups - 1:
                                # emit this head's softmax chain now so it
                                # overlaps the other head's last group
                                # fold the 2 partial-sum rows, then 1/denom
                                dsum = rp.tile([1, QB], F32, name="dsum", tag="dsum")
                                nc.vector.tensor_copy(
                                    dsum[:, :], psD[64 * h : 64 * h + 1, :]
                                )
                                nc.vector.scalar_tensor_tensor(
                                    out=dsum[:, :],
                                    in0=psD[64 * h + 32 : 64 * h + 33, :],
                                    scalar=1.0,
                                    in1=dsum[:, :],
                                    op0=mybir.AluOpType.mult,
                                    op1=mybir.AluOpType.add,
                                )
                                recip = rp.tile([1, QB], F32, name="recip", tag="recip")
                                nc.vector.reciprocal_approx_fast(recip[:, :], dsum[:, :])
                                # broadcast 1/denom across partitions on GpSimd
                                rb = rp.tile([128, QB], F32, name="rb", tag="rb")
                                nc.gpsimd.partition_broadcast(rb[:, :], recip[:, :])
                                o_base = (h * NQB + qb) * QB
                                nc.vector.tensor_mul(
                                    outt[:, o_base : o_base + QB], psO[h][:, :], rb[:, :]
                                )

                    # O-projection for this q-block (both heads ready); per
                    # s-tile, gather the 4 PSUM tiles into one SBUF tile
                    # (copies alternating DVE/ACT) and store with one DMA
                    for j in range(4):
                        st = qb * 4 + j
                        osb = op.tile([128, D], F32, name="osb", tag="osb")
                        for et in range(4):
                            psF = acc4.tile([128, QB], F32, name="psF", tag="acc4")
                            for h in range(NH):
                                o_base = (h * NQB + qb) * QB + j * 128
                                nc.tensor.matmul(
                                    psF[:, :],
                                    lhsT=outt[:, o_base : o_base + 128],
                                    rhs=wot_sb[:, h * D + et * 512 : h * D + (et + 1) * 512],
                                    start=(h == 0),
                                    stop=(h == NH - 1),
                                )
                            if qb == NQB - 1:
                                # tail: ACT is idle — split the drain copy
                                nc.vector.tensor_copy(
                                    osb[:, et * 512 : et * 512 + 256], psF[:, :256]
                                )
                                nc.scalar.copy(
                                    osb[:, et * 512 + 256 : (et + 1) * 512], psF[:, 256:]
                                )
                            else:
                                nc.vector.tensor_copy(
                                    osb[:, et * 512 : (et + 1) * 512], psF[:, :]
                                )
                        nc.sync.dma_start(
                            out=OUT[st * 128 : (st + 1) * 128, :], in_=osb[:, :]
                        )

                if is_causal:
                    # fused schedule: Q for block e first, then slice e's
                    # V/K projection units interleaved INTO attention block
                    # e's k-group stream as PE fill-work. Slice 0 instead
                    # runs V first — the V matmuls chase the chunked
                    # wv/x0 DMA stream from ~1 MB in, while Q's wq DMA only
                    # lands after the whole startup burst.
                    for sl in range(NSQ):
                        if sl + 1 < NSQ:
                            load_xe(sl + 1)
                        if sl == 0:
                            for j in range(SQ // 128):
                                emit_v_tile0(j)
                            for d in range(NH):
                                emit_qk0(wk_sb, 2, 2, d)
                            for d in range(NH):
                                emit_qk0(wq_sb, 0, 0, d)
                            attention_qb(0, [])
                            continue
                        for dt in range(NH):
                            emit_qk(sl, wq_sb, 0, 0, dt)
                        units = [
                            (lambda s=sl, j=j: emit_v_tile(s, j))
                            for j in range(SQ // 128)
                        ] + [
                            (lambda s=sl, d=d: emit_qk(s, wk_sb, 2, 2, d, on_dve=True))
                            for d in range(NH)
                        ]
                        attention_qb(sl, units)
                else:
                    for sl in range(NSQ):
                        if sl + 1 < NSQ:
                            load_xe(sl + 1)
                        proj_slice(sl)
                    for qb in range(NQB):
                        attention_qb(qb)
    nc.finalize()
    return nc


def _bf16(a: np.ndarray) -> np.ndarray:
    return np.ascontiguousarray(a.astype(ml_dtypes.bfloat16))


def make_in_maps(X, Wq, bq, Wk, bk, Wv, bv, Wo, is_causal: bool):
    x2d = np.asarray(X, dtype=np.float32).reshape(S, D)
    # xt2[sl*128+p, et*512+c] = X^T[et*128+p, sl*512+c]
    xt2 = _bf16(
        x2d.T.reshape(NET, 128, NSQ, SQ)
        .transpose(2, 1, 0, 3)
        .reshape(NSQ * 128, NET * SQ)
    )
    masks = np.zeros((128, 4 * QB), dtype=ml_dtypes.bfloat16)
    if is_causal:
        ki = np.arange(128)[:, None]
        qj = np.arange(QB)[None, :]
        for jj in range(4):
            masks[:, jj * QB : (jj + 1) * QB] = (128 * jj + ki <= qj).astype(
                ml_dtypes.bfloat16
            )

    def _pack_w(wT):  # [D, DL] -> [128, NET*DL]
        return _bf16(
            np.ascontiguousarray(wT).reshape(NET, 128, DL)
            .transpose(1, 0, 2)
            .reshape(128, NET * DL)
        )

    in_maps = []
    for c in range(NCORES):
        sl = slice(c * DL, (c + 1) * DL)
        wot = np.asarray(Wo)[:, sl].T  # [DL, D]
        wo2 = _bf16(wot.reshape(NH, 128, D).transpose(1, 0, 2).reshape(128, NH * D))
        in_maps.append(
            {
                "xt2": xt2,
                "wq2": _pack_w(np.asarray(Wq)[sl, :].T),
                "wk2": _pack_w(np.asarray(Wk)[sl, :].T),
                "wv2": _pack_w(np.asarray(Wv)[sl, :].T),
                "bqkc": np.ascontiguousarray(
                    np.stack(
                        [
                            np.asarray(bq, dtype=np.float32)[sl][:128],
                            np.asarray(bq, dtype=np.float32)[sl][128:],
                            np.asarray(bk, dtype=np.float32)[sl][:128],
                            np.asarray(bk, dtype=np.float32)[sl][128:],
                        ],
                        axis=1,
                    )
                ),
                "bvrow": _bf16(np.asarray(bv)[None, sl]),
                "wo2": wo2,
                "masks2": masks,
            }
        )
    return in_maps


_NC_CACHE: dict = {}


def _get_nc(is_causal: bool) -> bass.Bass:
    if is_causal not in _NC_CACHE:
        _NC_CACHE[is_causal] = build_nc(is_causal)
    return _NC_CACHE[is_causal]


def kernel(X, Wq, bq, Wk, bk, Wv, bv, Wo, bo, is_causal, **run_kwargs):
    causal = bool(int(np.asarray(is_causal)))
    nc = _get_nc(causal)
    in_maps = make_in_maps(X, Wq, bq, Wk, bk, Wv, bv, Wo, causal)
    res = run_bass_kernel_spmd(nc, in_maps, core_ids=list(range(NCORES)), **run_kwargs)
    out = np.asarray(bo, dtype=np.float32)[None, :].repeat(S, axis=0)
    for c in range(NCORES):
        out += res.results[c]["out"]
    return out.reshape(1, S, D)
